# revision 3
# baseline (speedup 1.0000x reference)
"""Multi-head attention (B=2, T=2048, d_model=1024, H=16, hd=64) on 8 Trainium2
NeuronCores.

Sharding: the 32 (batch, head) attention units are split as 4 consecutive heads
of one batch per core (core c -> batch c//4, heads 4*(c%4) .. 4*(c%4)+3). Each
core computes its own QKV projection slice, causal attention for its heads, and
a partial out-projection (its 256 rows of W_out). The host sums the 4 partials
per batch and adds b_out.

Device-side layout (everything flows transposed so no on-chip transposes are
needed until the attention output):
  qT/kT [hd, T]  <- lhsT=W slice, rhs=xT
  v     [T, hd]  (+ ones column for the row-sum trick)
  sT    [k, q]   <- lhsT=kT chunk, rhs=qT          (psum, fp32)
  E     [k, q]   <- exp(sT * 1/sqrt(hd)) on ScalarE (bf16)
  pv    [q, hd+1]<- lhsT=E chunk, rhs=[v|1]        (col hd = row sum)
  a     [q, hd]  = pv[:, :hd] * (1/pv[:, hd])      (per-partition scalar)
  aT    [hd, T]  via DRAM round-trip DMA transpose
  out  += aT.T @ W_out slice                        (partial, fp32)
"""

import math
import os
from contextlib import ExitStack
from dataclasses import dataclass

import numpy as np
import ml_dtypes

import concourse.bass as bass
import concourse.tile as tile
from concourse import bacc, mybir
from concourse import bass_utils

AF = mybir.ActivationFunctionType
ALU = mybir.AluOpType
DT = mybir.dt

N_CORES = 8
NEG = -1e9


@dataclass(frozen=True)
class Cfg:
    T: int = 2048        # sequence length
    DM: int = 1024       # d_model
    HD: int = 64         # head dim
    NH: int = 4          # heads per core
    mode: str = "causal"  # "causal" | "full" | "bias"
    mm: str = "bf16"     # matmul operand dtype: "bf16" | "f32r" | "f32"

    @property
    def NHD(self):
        return self.NH * self.HD          # qkv slice width per core

    @property
    def KC(self):
        return self.DM // 128             # contraction chunks for projections

    @property
    def MC(self):
        return self.NHD // 128            # qT/kT partition chunks

    @property
    def TC(self):
        return self.T // 128              # t chunks

    @property
    def QW(self):
        return min(512, self.T)           # q group width

    @property
    def QG(self):
        return self.T // self.QW

    @property
    def QT(self):
        return self.QW // 128             # q tiles per group

    @property
    def EB(self):
        return self.DM // 512             # out-proj free blocks

    @property
    def mmdt(self):
        return {"bf16": DT.bfloat16, "f32r": DT.float32r, "f32": DT.float32}[self.mm]

    @property
    def npmm(self):
        return ml_dtypes.bfloat16 if self.mm == "bf16" else np.float32


def build_program(cfg: Cfg):
    """Build + compile the SPMD single-core program. Returns (nc, input_names)."""
    c = cfg
    assert c.DM % 128 == 0 and c.NHD % 128 == 0 and c.T % 512 == 0
    nc = bacc.Bacc("TRN2", target_bir_lowering=False, debug=False,
                   num_devices=N_CORES)
    f32 = DT.float32
    bf16 = DT.bfloat16
    mmdt = c.mmdt

    xT = nc.dram_tensor("xT", [c.DM, c.T], mmdt, kind="ExternalInput").ap()
    wq = nc.dram_tensor("wq", [c.DM, c.NHD], mmdt, kind="ExternalInput").ap()
    wk = nc.dram_tensor("wk", [c.DM, c.NHD], mmdt, kind="ExternalInput").ap()
    wv = nc.dram_tensor("wv", [c.DM, c.NHD], mmdt, kind="ExternalInput").ap()
    bq = nc.dram_tensor("bq", [128, c.MC], f32, kind="ExternalInput").ap()
    bk = nc.dram_tensor("bk", [128, c.MC], f32, kind="ExternalInput").ap()
    bvb = nc.dram_tensor("bvb", [128, c.NHD], f32, kind="ExternalInput").ap()
    wo = nc.dram_tensor("wo", [c.NHD, c.DM], mmdt, kind="ExternalInput").ap()
    maskb = None
    if c.mode == "bias":
        # additive bias, transposed: maskb[k, q]
        maskb = nc.dram_tensor("maskb", [c.T, c.T], f32, kind="ExternalInput").ap()
    out = nc.dram_tensor("out", [c.T, c.DM], f32, kind="ExternalOutput").ap()

    with tile.TileContext(nc) as tc, ExitStack() as ctx:
        _body(ctx, tc, c, xT, wq, wk, wv, bq, bk, bvb, wo, maskb, out)
    nc.compile()
    names = ["xT", "wq", "wk", "wv", "bq", "bk", "bvb", "wo"]
    if c.mode == "bias":
        names.append("maskb")
    return nc, names


def _body(ctx, tc, c: Cfg, xT, wq, wk, wv, bq, bk, bvb, wo, maskb, out):
    nc = tc.nc
    f32 = DT.float32
    bf16 = DT.bfloat16
    mmdt = c.mmdt
    causal = c.mode == "causal"
    scale = 1.0 / math.sqrt(c.HD)

    const = ctx.enter_context(tc.tile_pool(name="const", bufs=1))
    big = ctx.enter_context(tc.tile_pool(name="big", bufs=1))
    epool = ctx.enter_context(tc.tile_pool(name="E", bufs=c.TC))
    rpool = ctx.enter_context(tc.tile_pool(name="r", bufs=8))
    ps_mm = ctx.enter_context(tc.tile_pool(name="psmm", bufs=4, space="PSUM"))
    ps_pv = ctx.enter_context(tc.tile_pool(name="pspv", bufs=4, space="PSUM"))
    dramp = ctx.enter_context(tc.tile_pool(name="dram", bufs=1, space="DRAM"))
    bias_pool = None
    if c.mode == "bias":
        bias_pool = ctx.enter_context(tc.tile_pool(name="maskb", bufs=4))

    # ---- load inputs to SBUF ----
    xT_sb = big.tile([128, c.KC, c.T], mmdt, tag="xT")
    xTd = xT.rearrange("(c p) t -> c p t", p=128)
    for k in range(c.KC):
        nc.sync.dma_start(out=xT_sb[:, k, :], in_=xTd[k])

    w_sbs = []
    for nm, w in (("wq", wq), ("wk", wk), ("wv", wv)):
        w_sb = big.tile([128, c.KC, c.NHD], mmdt, tag=nm)
        wd = w.rearrange("(c p) n -> c p n", p=128)
        for k in range(c.KC):
            nc.sync.dma_start(out=w_sb[:, k, :], in_=wd[k])
        w_sbs.append(w_sb)
    wq_sb, wk_sb, wv_sb = w_sbs

    wo_sb = big.tile([128, c.MC, c.DM], mmdt, tag="wo")
    wod = wo.rearrange("(c p) n -> c p n", p=128)
    for k in range(c.MC):
        nc.sync.dma_start(out=wo_sb[:, k, :], in_=wod[k])

    bq_sb = const.tile([128, c.MC], f32, tag="bq")
    nc.sync.dma_start(out=bq_sb[:], in_=bq)
    bk_sb = const.tile([128, c.MC], f32, tag="bk")
    nc.sync.dma_start(out=bk_sb[:], in_=bk)
    bvb_sb = const.tile([128, c.NHD], f32, tag="bvb")
    nc.sync.dma_start(out=bvb_sb[:], in_=bvb)

    # causal mask block for diagonal tiles: tri[k, j] = 0 if j >= k else NEG
    tri = const.tile([128, 128], f32, tag="tri")
    nc.gpsimd.memset(tri[:], 0.0)
    nc.gpsimd.affine_select(
        out=tri[:], in_=tri[:],
        compare_op=ALU.is_ge, fill=NEG,
        base=0, channel_multiplier=-1, pattern=[[1, 128]],
    )

    # ---- QKV projections ----
    qT_sb = big.tile([128, c.MC, c.T], mmdt, tag="qT")
    kT_sb = big.tile([128, c.MC, c.T], mmdt, tag="kT")
    NB = c.T // 512
    for w_sb, b_sb, dst in ((wq_sb, bq_sb, qT_sb), (wk_sb, bk_sb, kT_sb)):
        for m in range(c.MC):
            for n in range(NB):
                ps = ps_mm.tile([128, 512], f32, tag="mm")
                for k in range(c.KC):
                    nc.tensor.matmul(
                        ps[:],
                        lhsT=w_sb[:, k, m * 128:(m + 1) * 128],
                        rhs=xT_sb[:, k, n * 512:(n + 1) * 512],
                        start=(k == 0), stop=(k == c.KC - 1),
                    )
                nc.scalar.activation(
                    dst[:, m, n * 512:(n + 1) * 512], ps[:],
                    AF.Identity, bias=b_sb[:, m:m + 1], scale=1.0,
                )

    # v in normal layout, augmented with a ones column per head
    HD1 = c.HD + 1
    v_sb = big.tile([128, c.TC, c.NH, HD1], bf16, tag="v")
    nc.vector.memset(v_sb[:, :, :, c.HD:HD1], 1.0)
    for t in range(c.TC):
        ps = ps_mm.tile([128, 512], f32, tag="mm")
        for k in range(c.KC):
            nc.tensor.matmul(
                ps[:, 0:c.NHD],
                lhsT=xT_sb[:, k, t * 128:(t + 1) * 128],
                rhs=wv_sb[:, k, :],
                start=(k == 0), stop=(k == c.KC - 1),
            )
        nc.vector.tensor_tensor(
            out=v_sb[:, t, :, 0:c.HD],
            in0=ps[:, 0:c.NHD].rearrange("p (h d) -> p h d", d=c.HD),
            in1=bvb_sb.rearrange("p (h d) -> p h d", d=c.HD),
            op=ALU.add,
        )

    # ---- attention per head ----
    a_sb = big.tile([128, c.TC, c.NH, c.HD], bf16, tag="a")
    for h in range(c.NH):
        m, p0 = h // 2, (h % 2) * 64
        for g in range(c.QG):
            kmax = (g + 1) * c.QT if causal else c.TC
            etiles = []
            for kc in range(kmax):
                ps = ps_mm.tile([128, 512], f32, tag="mm")
                nc.tensor.matmul(
                    ps[:, 0:c.QW],
                    lhsT=kT_sb[p0:p0 + 64, m, kc * 128:(kc + 1) * 128],
                    rhs=qT_sb[p0:p0 + 64, m, g * c.QW:(g + 1) * c.QW],
                    start=True, stop=True,
                )
                if causal:
                    off = (kc - g * c.QT) * 128
                    if off >= 0:
                        nc.vector.tensor_tensor(
                            out=ps[:, off:off + 128],
                            in0=ps[:, off:off + 128], in1=tri[:], op=ALU.add,
                        )
                elif c.mode == "bias":
                    mb = bias_pool.tile([128, c.QW], f32, tag="mb")
                    nc.sync.dma_start(
                        out=mb[:],
                        in_=maskb[kc * 128:(kc + 1) * 128,
                                  g * c.QW:(g + 1) * c.QW],
                    )
                    nc.vector.tensor_tensor(
                        out=ps[:, 0:c.QW], in0=ps[:, 0:c.QW], in1=mb[:],
                        op=ALU.add,
                    )
                et = epool.tile([128, c.QW], bf16, tag="E")
                nc.scalar.activation(et[:], ps[:, 0:c.QW], AF.Exp, scale=scale)
                etiles.append(et)
            for j in range(c.QT):
                qt = g * c.QT + j
                kn = qt + 1 if causal else c.TC
                psv = ps_pv.tile([128, HD1], f32, tag="pv")
                for kc in range(kn):
                    nc.tensor.matmul(
                        psv[:],
                        lhsT=etiles[kc][:, j * 128:(j + 1) * 128],
                        rhs=v_sb[:, kc, h, :],
                        start=(kc == 0), stop=(kc == kn - 1),
                    )
                r = rpool.tile([128, 1], f32, tag="r")
                nc.vector.reciprocal(r[:], psv[:, c.HD:HD1])
                nc.vector.tensor_scalar_mul(
                    a_sb[:, qt, h, :], psv[:, 0:c.HD], r[:, 0:1],
                )

    # ---- transpose a -> aT via DRAM round trip ----
    a_dram = dramp.tile([c.T, c.NHD], bf16, tag="adram")
    nc.sync.dma_start(
        out=a_dram.rearrange("(t p) n -> p t n", p=128),
        in_=a_sb.rearrange("p t h d -> p t (h d)"),
    )
    aT_sb = big.tile([128, c.MC, c.T], bf16, tag="aT")
    for ci in range(c.MC):
        nc.sync.dma_start(
            out=aT_sb[:, ci, :],
            in_=a_dram[:, ci * 128:(ci + 1) * 128],
            transpose=True,
        )

    # ---- partial out-projection ----
    ostage = ctx.enter_context(tc.tile_pool(name="ostage", bufs=3))
    for t in range(c.TC):
        for eb in range(c.EB):
            ps = ps_mm.tile([128, 512], f32, tag="mm")
            for ci in range(c.MC):
                nc.tensor.matmul(
                    ps[:],
                    lhsT=aT_sb[:, ci, t * 128:(t + 1) * 128],
                    rhs=wo_sb[:, ci, eb * 512:(eb + 1) * 512],
                    start=(ci == 0), stop=(ci == c.MC - 1),
                )
            ot = ostage.tile([128, 512], f32, tag="o")
            nc.scalar.copy(ot[:], ps[:])
            nc.sync.dma_start(
                out=out[t * 128:(t + 1) * 128, eb * 512:(eb + 1) * 512],
                in_=ot[:],
            )


# ---------------------------------------------------------------------------
# host side
# ---------------------------------------------------------------------------

_CACHE: dict = {}


def _get_program(cfg: Cfg):
    key = cfg
    if key not in _CACHE:
        _CACHE[key] = build_program(cfg)
    return _CACHE[key]


def _mask_mode(mask: np.ndarray, T: int) -> str:
    m = (np.asarray(mask).reshape(T, T) != 0)
    if m.all():
        return "full"
    if np.array_equal(m, np.tril(np.ones((T, T), dtype=bool))):
        return "causal"
    return "bias"


def make_in_maps(cfg: Cfg, x, W_qkv, b_qkv, W_out, mask=None):
    """Slice full inputs into the 8 per-core input dicts."""
    c = cfg
    npmm = c.npmm
    B = x.shape[0]
    n_hg = N_CORES // B                      # head groups per batch
    in_maps = []
    maskb = None
    if c.mode == "bias":
        m = (np.asarray(mask).reshape(c.T, c.T) != 0)
        maskb = np.where(m, np.float32(0), np.float32(NEG)).T.copy()
    for core in range(N_CORES):
        b, hg = divmod(core, n_hg)
        col0 = hg * c.NHD
        xT = np.ascontiguousarray(x[b].T).astype(npmm)
        wq_ = np.ascontiguousarray(W_qkv[:, 0 * c.DM + col0:0 * c.DM + col0 + c.NHD]).astype(npmm)
        wk_ = np.ascontiguousarray(W_qkv[:, 1 * c.DM + col0:1 * c.DM + col0 + c.NHD]).astype(npmm)
        wv_ = np.ascontiguousarray(W_qkv[:, 2 * c.DM + col0:2 * c.DM + col0 + c.NHD]).astype(npmm)
        bq_ = np.ascontiguousarray(
            b_qkv[0 * c.DM + col0:0 * c.DM + col0 + c.NHD].reshape(c.MC, 128).T
        ).astype(np.float32)
        bk_ = np.ascontiguousarray(
            b_qkv[1 * c.DM + col0:1 * c.DM + col0 + c.NHD].reshape(c.MC, 128).T
        ).astype(np.float32)
        bv_ = b_qkv[2 * c.DM + col0:2 * c.DM + col0 + c.NHD].astype(np.float32)
        bvb_ = np.ascontiguousarray(np.broadcast_to(bv_, (128, c.NHD)))
        wo_ = np.ascontiguousarray(W_out[col0:col0 + c.NHD, :]).astype(npmm)
        im = dict(xT=xT, wq=wq_, wk=wk_, wv=wv_, bq=bq_, bk=bk_, bvb=bvb_,
                  wo=wo_)
        if c.mode == "bias":
            im["maskb"] = maskb
        in_maps.append(im)
    return in_maps


def run_sharded(cfg: Cfg, x, W_qkv, b_qkv, W_out, b_out, mask=None, **kw):
    """Run the SPMD program on 8 cores and assemble the full output."""
    nc, _names = _get_program(cfg)
    in_maps = make_in_maps(cfg, x, W_qkv, b_qkv, W_out, mask)
    res = bass_utils.run_bass_kernel_spmd(
        nc, in_maps, core_ids=list(range(N_CORES)), **kw,
    )
    outs = [r["out"] for r in res.results]
    B = x.shape[0]
    n_hg = N_CORES // B
    y = np.stack([
        np.sum(outs[b * n_hg:(b + 1) * n_hg], axis=0) for b in range(B)
    ]) + b_out.astype(np.float32)
    return y.astype(np.float32), res


def kernel(x, W_qkv, b_qkv, W_out, b_out, mask):
    x = np.asarray(x, dtype=np.float32)
    W_qkv = np.asarray(W_qkv, dtype=np.float32)
    b_qkv = np.asarray(b_qkv, dtype=np.float32)
    W_out = np.asarray(W_out, dtype=np.float32)
    b_out = np.asarray(b_out, dtype=np.float32)
    B, T, DM = x.shape
    mode = _mask_mode(mask, T)
    cfg = Cfg(T=T, DM=DM, mode=mode, mm=os.environ.get("MHA_MM_DT", "bf16"))
    y, _ = run_sharded(cfg, x, W_qkv, b_qkv, W_out, b_out, mask)
    return y
